# revision 25
# baseline (speedup 1.0000x reference)
"""GAT layer (edge softmax + scatter aggregation) on 8 Trainium2 NeuronCores.

Strategy
--------
Edges are partitioned by *destination* node (8 contiguous node ranges, one per
core), sorted by dst on the host.  This makes the segment softmax and the
scatter-sum aggregation fully core-local: no reduction collectives at all.

Per core:
  Phase A  - q/v projections for ALL nodes (replicated compute; collectives on
             this part are slower than recomputing), k projection for the
             local node slice only.  Tables stored fp16 in DRAM.
  Phase B  - stream the core's edges (sorted by dst) in 128-edge subtiles
             grouped under 128-node dst blocks:
               * indirect-DMA gather of [q||v] rows by src and k rows by dst
               * DVE: s = per-head reduce(q*k);  ACT: w = exp(s/sqrt(D))
                 (softmax max-subtraction is skipped: it is mathematically a
                 no-op and scores here are O(+-8), far from fp32 overflow)
               * one-hot dst mask via tensor_scalar(is_equal) against an iota
                 row; PE matmuls accumulate both the weighted message sums
                 ft^T[d, node] and the softmax denominators den[h, node] in
                 PSUM across the block's subtiles.
             Block tail: reciprocal of den, normalize, final (head-summed)
             Wscale projection, residual add, store.

The head-sum of the reference's `rst.sum(axis=1)` is folded into the weight:
W2[i,j] = sum_h Wscale[h*IN+i, j], so the final projection is a single
128x128 matmul per node tile.
"""

import math
from contextlib import ExitStack

import numpy as np

import bass_rust
import concourse.bacc as bacc
import concourse.bass as bass
import concourse.mybir as mybir
import concourse.tile as tile
from concourse.bass import IndirectOffsetOnAxis
from concourse.bass_utils import run_bass_kernel_spmd

P = 128          # partitions / tile edge
H = 4            # heads
D = 32           # per-head dim
NCORES = 8
BATCH = 4        # subtiles batched per DVE/ACT op

F16 = mybir.dt.float16
F32 = mybir.dt.float32
I32 = mybir.dt.int32

# constant subtracted inside exp() so weights stay in fp16 range
# (softmax is shift-invariant; scores here are ~N(0, 1.3), range +-13)
EXP_SHIFT = -6.0


# --------------------------------------------------------------------------
# host-side prep: sharding, sorting, padding, weight folding
# --------------------------------------------------------------------------

def _prep(feature, Wq, Wk, Wv, Wscale, src, dst):
    N, IN = feature.shape
    E = src.shape[0]
    HD = Wq.shape[0]
    assert IN == H * D and HD == H * D

    nodes_pc = -(-N // (NCORES * P)) * P          # nodes per core, 128-aligned
    npad = nodes_pc * NCORES
    T = npad // P                                  # total node tiles
    blocks = nodes_pc // P                         # node tiles per core

    featp = np.zeros((npad, IN), np.float32)
    featp[:N] = np.asarray(feature, np.float32)
    # [tile, feat, node] so each tile is a ready-to-load matmul lhsT
    featT_t = np.ascontiguousarray(
        featp.reshape(T, P, IN).transpose(0, 2, 1)).astype(np.float16)
    featT_loc = featT_t.reshape(NCORES, blocks, IN, P)
    feat_res = np.ascontiguousarray(featp.reshape(NCORES, nodes_pc, IN))

    wqvT = np.ascontiguousarray(
        np.concatenate([np.asarray(Wq, np.float32).T,
                        np.asarray(Wv, np.float32).T], axis=1)).astype(np.float16)
    wkT = np.ascontiguousarray(np.asarray(Wk, np.float32).T).astype(np.float16)
    # fold the head-sum of the output projection into the weight
    W2 = np.asarray(Wscale, np.float32).reshape(H, IN, HD).sum(0)   # [i, j]
    w2T = np.ascontiguousarray(W2.T).astype(np.float16)             # [j, i]

    e4 = np.zeros((H, P), np.float16)
    for h in range(H):
        e4[h, h * D:(h + 1) * D] = 1.0
    iotaf = np.tile(np.arange(P, dtype=np.float16), (P, 1))
    iotaf = np.ascontiguousarray(iotaf)

    # ---- edge sharding: sort by dst, bucket into 128-node blocks ----
    src64 = np.asarray(src, np.int64)
    dst64 = np.asarray(dst, np.int64)
    order = np.argsort(dst64, kind="stable")
    src_s = src64[order].astype(np.int32)
    dst_s = dst64[order]
    gblk = dst_s // P                                    # global block id
    counts = np.bincount(gblk, minlength=T)
    offs = np.zeros(T + 1, np.int64)
    offs[1:] = np.cumsum(counts)
    cnts2 = counts.reshape(NCORES, blocks)
    # SPMD: one program for all cores -> per-block subtile count is the max
    # over cores (>=1 so PSUM always gets its start=True matmul)
    S = np.maximum(1, -(-cnts2 // P)).max(axis=0).astype(np.int64)
    nsub = int(S.sum())
    subo = np.zeros(blocks + 1, np.int64)
    subo[1:] = np.cumsum(S)

    srcidx = np.zeros((NCORES, nsub * P), np.int32)
    kidx = np.zeros((NCORES, nsub * P), np.int32)
    dstloc = np.full((NCORES, nsub * P), 255.0, np.float32)

    c_e = gblk // blocks
    b_e = gblk % blocks
    rank = np.arange(E, dtype=np.int64) - offs[gblk]
    slot = subo[b_e] * P + rank
    srcidx[c_e, slot] = src_s
    kidx[c_e, slot] = (dst_s - c_e * nodes_pc).astype(np.int32)
    dstloc[c_e, slot] = (dst_s % P).astype(np.float32)

    srcidx = np.ascontiguousarray(
        srcidx.reshape(NCORES, nsub, P).transpose(0, 2, 1))
    kidx = np.ascontiguousarray(
        kidx.reshape(NCORES, nsub, P).transpose(0, 2, 1))
    dstloc = np.ascontiguousarray(
        dstloc.reshape(NCORES, nsub, P).transpose(0, 2, 1))

    meta = dict(N=N, IN=IN, E=E, npad=npad, T=T, blocks=blocks,
                nodes_pc=nodes_pc, nsub=nsub, S=[int(x) for x in S])
    in_maps = []
    for c in range(NCORES):
        in_maps.append({
            "featT": featT_t,
            "featTloc": np.ascontiguousarray(featT_loc[c]),
            "featres": feat_res[c],
            "wqvT": wqvT,
            "wkT": wkT,
            "w2T": w2T,
            "e4": e4,
            "iotaf": iotaf,
            "srcidx": srcidx[c],
            "kidx": kidx[c],
            "dstloc": dstloc[c],
        })
    return in_maps, meta


# --------------------------------------------------------------------------
# device program (SPMD, one Bass for all 8 cores)
# --------------------------------------------------------------------------

def _build(meta):
    T = meta["T"]
    blocks = meta["blocks"]
    nodes_pc = meta["nodes_pc"]
    nsub = meta["nsub"]
    npad = meta["npad"]
    S = meta["S"]

    nc = bacc.Bacc()

    featT = nc.dram_tensor("featT", [T, P, P], F16, kind="ExternalInput")
    featTloc = nc.dram_tensor("featTloc", [blocks, P, P], F16, kind="ExternalInput")
    featres = nc.dram_tensor("featres", [nodes_pc, P], F32, kind="ExternalInput")
    wqvT = nc.dram_tensor("wqvT", [P, 2 * P], F16, kind="ExternalInput")
    wkT = nc.dram_tensor("wkT", [P, P], F16, kind="ExternalInput")
    w2T = nc.dram_tensor("w2T", [P, P], F16, kind="ExternalInput")
    e4 = nc.dram_tensor("e4", [H, P], F16, kind="ExternalInput")
    iotaf = nc.dram_tensor("iotaf", [P, P], F16, kind="ExternalInput")
    srcidx = nc.dram_tensor("srcidx", [P, nsub], I32, kind="ExternalInput")
    kidx = nc.dram_tensor("kidx", [P, nsub], I32, kind="ExternalInput")
    dstloc = nc.dram_tensor("dstloc", [P, nsub], F32, kind="ExternalInput")
    out_t = nc.dram_tensor("out", [nodes_pc, P], F32, kind="ExternalOutput")

    qvtab = nc.dram_tensor("qvtab", [npad, 2 * P], F16)
    ktab = nc.dram_tensor("ktab", [nodes_pc, P], F16)

    mult = mybir.AluOpType.mult
    scale = float(D) ** -0.5

    with ExitStack() as ctx:
        tc = ctx.enter_context(tile.TileContext(nc))
        const = ctx.enter_context(tc.tile_pool(name="const", bufs=1))

        wqv_sb = const.tile([P, 2 * P], F16)
        nc.sync.dma_start(out=wqv_sb[:], in_=wqvT[:])
        wk_sb = const.tile([P, P], F16)
        nc.sync.dma_start(out=wk_sb[:], in_=wkT[:])
        w2_sb = const.tile([P, P], F16)
        nc.sync.dma_start(out=w2_sb[:], in_=w2T[:])
        e4_sb = const.tile([H, P], F16)
        nc.sync.dma_start(out=e4_sb[:], in_=e4[:])
        iota_sb = const.tile([P, P], F16)
        nc.sync.dma_start(out=iota_sb[:], in_=iotaf[:])
        six_sb = const.tile([P, nsub], I32)
        nc.sync.dma_start(out=six_sb[:], in_=srcidx[:])
        kix_sb = const.tile([P, nsub], I32)
        nc.sync.dma_start(out=kix_sb[:], in_=kidx[:])
        dstl_sb = const.tile([P, nsub], F32)
        nc.sync.dma_start(out=dstl_sb[:], in_=dstloc[:])
        bias_sb = const.tile([P, 1], F32)
        nc.vector.memset(bias_sb[:], EXP_SHIFT)

        # phase-B pools allocated first so their SBUF/PSUM zones never
        # overlap the released phase-A pools (zone reuse would put >2 sync
        # waits on DMA instructions, which the DMA ISA cannot encode)
        pe = ctx.enter_context(tc.tile_pool(name="pe", bufs=3))
        tail = ctx.enter_context(tc.tile_pool(name="tail", bufs=8))

        # ---------------- Phase A: projections ----------------
        with tc.tile_pool(name="pa", bufs=16) as pa, \
             tc.tile_pool(name="paps", bufs=2, space="PSUM") as paps:
            tab_writes = []
            for t in range(T):
                ft = pa.tile([P, P], F16, tag="ft")
                tab_writes.append(nc.gpsimd.dma_start(out=ft[:], in_=featT[t]))
                qv_ps = paps.tile([P, 2 * P], F32, tag="qv_ps")
                nc.tensor.matmul(qv_ps[:], lhsT=ft[:], rhs=wqv_sb[:],
                                 start=True, stop=True)
                qv_sb = pa.tile([P, 2 * P], F16, tag="qv_sb")
                tab_writes.append(nc.any.tensor_copy(qv_sb[:], qv_ps[:]))
                tab_writes.append(
                    nc.sync.dma_start(out=qvtab[t * P:(t + 1) * P, :],
                                      in_=qv_sb[:]))
            for b in range(blocks):
                ftl = pa.tile([P, P], F16, tag="ft")
                tab_writes.append(nc.gpsimd.dma_start(out=ftl[:], in_=featTloc[b]))
                k_ps = paps.tile([P, P], F32, tag="k_ps")
                nc.tensor.matmul(k_ps[:], lhsT=ftl[:], rhs=wk_sb[:],
                                 start=True, stop=True)
                k_sb = pa.tile([P, P], F16, tag="k_sb")
                tab_writes.append(nc.any.tensor_copy(k_sb[:], k_ps[:]))
                tab_writes.append(
                    nc.sync.dma_start(out=ktab[b * P:(b + 1) * P, :],
                                      in_=k_sb[:]))

        # ---------------- Phase B: edges ----------------
        # Pool-engine fence: absorb all table-write DMA completions here so
        # the gathers below don't each carry >2 sync waits (HW DMA limit)
        fence = nc.gpsimd.engine_nop()
        fence_sp = nc.sync.nop()
        for w in tab_writes:
            tile.add_dep_helper(fence.ins, w.ins, reason="qvtab/ktab fence")
            tile.add_dep_helper(fence_sp.ins, w.ins, reason="sp lane fence")
        tc.strict_bb_all_engine_barrier()
        agg_ps = ctx.enter_context(tc.tile_pool(name="aggps", bufs=2, space="PSUM"))
        tail_ps = ctx.enter_context(tc.tile_pool(name="tailps", bufs=2, space="PSUM"))

        gj = 0
        for b in range(blocks):
            ftT_ps = agg_ps.tile([P, P], F32, tag="ftT")
            den_ps = agg_ps.tile([H, P], F32, tag="den")
            nsub_b = S[b]
            first = True
            for j0 in range(0, nsub_b, BATCH):
                nb = min(BATCH, nsub_b - j0)
                qv = pe.tile([P, nb, 2 * P], F16, tag="qv")
                kg = pe.tile([P, nb, P], F16, tag="kg")
                for jj in range(nb):
                    nc.gpsimd.indirect_dma_start(
                        out=qv[:, jj, :], out_offset=None, in_=qvtab[:],
                        in_offset=IndirectOffsetOnAxis(
                            ap=six_sb[:, gj + jj:gj + jj + 1], axis=0))
                    nc.gpsimd.indirect_dma_start(
                        out=kg[:, jj, :], out_offset=None, in_=ktab[:],
                        in_offset=IndirectOffsetOnAxis(
                            ap=kix_sb[:, gj + jj:gj + jj + 1], axis=0))
                qk = pe.tile([P, nb, P], F16, tag="qk")
                nc.vector.tensor_tensor(
                    out=qk[:], in0=qv[:, :, 0:P], in1=kg[:], op=mult)
                s4 = pe.tile([P, nb * H], F32, tag="s4")
                nc.vector.tensor_reduce(
                    out=s4[:],
                    in_=qk[:].rearrange("p n (h d) -> p (n h) d", d=D),
                    axis=mybir.AxisListType.X, op=mybir.AluOpType.add)
                # w broadcast-expanded over d while applying exp on ACT
                wexp = pe.tile([P, nb, H, D], F16, tag="wexp")
                # constant shift keeps exp() in fp16 range; softmax is
                # shift-invariant so the result is unchanged
                nc.scalar.activation(
                    out=wexp[:].rearrange("p n h d -> p (n h) d"),
                    in_=s4[:].to_broadcast([P, nb * H, D]),
                    func=mybir.ActivationFunctionType.Exp, scale=scale,
                    bias=bias_sb[:, 0:1])
                masks = pe.tile([P, nb, P], F16, tag="masks")
                for jj in range(nb):
                    nc.vector.tensor_scalar(
                        out=masks[:, jj, :], in0=iota_sb[:],
                        scalar1=dstl_sb[:, gj + jj:gj + jj + 1], scalar2=None,
                        op0=mybir.AluOpType.is_equal)
                vp = pe.tile([P, nb, P], F16, tag="vp")
                nc.vector.tensor_tensor(
                    out=vp[:], in0=qv[:, :, P:2 * P],
                    in1=wexp[:].rearrange("p n h d -> p n (h d)"), op=mult)
                for jj in range(nb):
                    last = (j0 + jj == nsub_b - 1)
                    nc.tensor.matmul(ftT_ps[:], lhsT=vp[:, jj, :],
                                     rhs=masks[:, jj, :], start=first, stop=last)
                    nc.tensor.matmul(den_ps[:], lhsT=wexp[:, jj, :, 0],
                                     rhs=masks[:, jj, :], start=first, stop=last)
                    first = False
                gj += nb

            denm = tail.tile([H, P], F32, tag="denm")
            nc.vector.tensor_scalar(
                out=denm[:], in0=den_ps[:], scalar1=2e-5, scalar2=None,
                op0=mybir.AluOpType.max)
            recip = tail.tile([H, P], F16, tag="recip")
            with nc.allow_low_precision("recip fits fp16: den clamped >=2e-5"):
                nc.vector.reciprocal(recip[:], denm[:])
            rexp_ps = tail_ps.tile([P, P], F32, tag="rexp")
            nc.tensor.matmul(rexp_ps[:], lhsT=e4_sb[:], rhs=recip[:],
                             start=True, stop=True)
            rexp_sb = tail.tile([P, P], F16, tag="rexp_sb")
            nc.scalar.copy(rexp_sb[:], rexp_ps[:])
            ftn = tail.tile([P, P], F16, tag="ftn")
            nc.vector.tensor_tensor(
                out=ftn[:], in0=ftT_ps[:], in1=rexp_sb[:], op=mult)
            y_ps = tail_ps.tile([P, P], F32, tag="y")
            nc.tensor.matmul(y_ps[:], lhsT=ftn[:], rhs=w2_sb[:],
                             start=True, stop=True)
            featr = tail.tile([P, P], F32, tag="featr")
            nc.gpsimd.dma_start(out=featr[:], in_=featres[b * P:(b + 1) * P, :])
            yout = tail.tile([P, P], F32, tag="yout")
            nc.vector.tensor_add(out=yout[:], in0=y_ps[:], in1=featr[:])
            nc.scalar.dma_start(out=out_t[b * P:(b + 1) * P, :], in_=yout[:])

    nc.finalize()
    return nc


def _cap_dma_waits(nc, max_keep=1, nop_chunk=1):
    """walrus's DMA DIRECT2D pseudo-instruction encodes at most 2 sync
    waits; Tile occasionally emits more.  Move the excess onto same-engine
    NOPs placed just before the DMA -- waits are executed by the issuing
    sequencer either way, so this is semantics-preserving."""
    uid = 0
    for f in nc.m.functions:
        for bb in f.blocks:
            il = bb.instructions
            out = []
            changed = False
            for ins in il:
                si = ins.sync_info
                if (isinstance(ins, mybir.InstDMACopy) and si is not None
                        and si.on_wait and len(si.on_wait) > max_keep):
                    waits = list(si.on_wait)
                    excess, keep = waits[:-max_keep], waits[-max_keep:]
                    for i0 in range(0, len(excess), nop_chunk):
                        nop = mybir.InstNoOp(
                            name=f"Z-waitsplit-{uid}", ins=[], outs=[])
                        uid += 1
                        nop.engine = ins.engine
                        nop.sync_info = bass_rust.SyncInfo(
                            on_wait=excess[i0:i0 + nop_chunk], on_update=[])
                        out.append(nop)
                    ins.sync_info = bass_rust.SyncInfo(
                        on_wait=keep, on_update=list(si.on_update or []))
                    changed = True
                out.append(ins)
            if changed:
                bb.instructions = out


# --------------------------------------------------------------------------
# entry point
# --------------------------------------------------------------------------

def _run(inputs, trace=False):
    in_maps, meta = _prep(**inputs)
    nc = _build(meta)
    res = run_bass_kernel_spmd(nc, in_maps, list(range(NCORES)))
    outs = np.concatenate([res.results[i]["out"] for i in range(NCORES)], axis=0)
    return outs[:meta["N"]].astype(np.float32), res


def kernel(**inputs):
    out, _ = _run(inputs, trace=False)
    return out
